# revision 17
# baseline (speedup 1.0000x reference)
"""CropAndResize (tf.image.crop_and_resize semantics, bilinear, extrap=0)
Trainium2 Bass kernel, data-parallel over 8 NeuronCores.

Full inputs:  img (4,512,64,64) f32, rois (4,300,4) f32, input_image (4,3,1024,1024) f32
Full output:  (4,300,512,7,7) f32

Sharding: core c handles image n = c//2 and roi slice
[(c%2)*150 : (c%2)*150+150].

Host prep (per core, cheap O(KB) numpy on the 4-number-per-roi boxes):
  - img is transposed to row-major [hw, c] fp16 with a channel permutation
    pi(c) = (c//4) + 128*(c%4), so the transpose-mode dma_gather lands
    channel 4p+j on partition p, slot j. That makes the final output DMA
    descriptor (j,py,px) = 784B contiguous (full DMA bandwidth, no <512B
    penalty).
  - bilinear corner indices (wrapped int16 [16,*] layout, replicated to 128
    partitions) and fp16 corner weights (dense j = r*49+pt rows for the PE
    ones-broadcast) are computed from the rois in f32, matching the
    reference arithmetic step for step.

Device program per batch (sizes 18x8 + 6 rois; per-corner num_idxs is
n*49 padded to a multiple of 128, kept under the ~1024-idx dma_gather cap):
  1. four per-corner dma_gathers (1KiB rows) from DRAM img.
  2. per corner: PE ones-matmul broadcasts the weight row to 128
     partitions (PSUM), Act copies PSUM -> fp16 SBUF.
  3. DVE blend: ob[j] = sum_k T_k[j] * w_k  (fp16, 2x DVE mode, weight
     broadcast across the 4 channel slots via a stride-0 free dim).
  4. Act casts fp16 -> f32 into the (r, j, s) output layout.
  5. one HWDGE DMA per half-batch writes out[r, 4p+j, py, px]
     (784B descriptors, full DMA bandwidth).
"""

import os
import sys

import numpy as np

_RL_REPO_CANDIDATES = ["/opt/trn_rl_repo", "/root/.axon_site/_ro/trn_rl_repo"]
for _p in _RL_REPO_CANDIDATES:
    if os.path.isdir(_p) and _p not in sys.path:
        sys.path.insert(0, _p)

# ---------------------------------------------------------------- constants
N_CORES = 8
N, C, H, W = 4, 512, 64, 64
B = 300
POOL = 7
PTS = POOL * POOL      # 49
IH, IW = 1024.0, 1024.0
HW = H * W             # 4096
R_CORE = B // 2        # 150 rois per core


def _mk_batches():
    # (roi_start, n_rois, nidxc): per-corner idx count is n*49 padded to a
    # multiple of 128; the dma_gather HW caps num_idxs below 1024, so 18
    # rois (882 -> 896) is the densest clean batch. 8x18 + 6 = 150.
    out = []
    r0 = 0
    for nr in [18] * 8 + [6]:
        nidxc = -(-(nr * PTS) // 128) * 128
        out.append((r0, nr, nidxc))
        r0 += nr
    return out


BATCHES = _mk_batches()
NB = len(BATCHES)
IDX_COLS = sum(4 * nx // 16 for _, _, nx in BATCHES)   # wrapped idx columns
WR_LEN = sum(4 * nx for _, _, nx in BATCHES)           # weight row length
RB_MAX = max(nr for _, nr, _ in BATCHES)
NXM = max(nx for _, _, nx in BATCHES)   # largest per-corner idx count

_prog_cache = {}


def _build_program():
    import concourse.bass as bass
    import concourse.bacc as bacc
    import concourse.mybir as mybir
    import concourse.tile as tile

    f32 = mybir.dt.float32
    f16 = mybir.dt.float16
    i16 = mybir.dt.int16

    nc = bacc.Bacc("TRN2", target_bir_lowering=False, debug=False,
                   num_devices=N_CORES)

    img_in = nc.dram_tensor("img", (HW, C), f16, kind="ExternalInput")
    idx_in = nc.dram_tensor("idx", (128, IDX_COLS), i16, kind="ExternalInput")
    wr_in = nc.dram_tensor("wrows", (1, WR_LEN), f16, kind="ExternalInput")
    ones_in = nc.dram_tensor("ones", (1, 128), f16, kind="ExternalInput")
    out_t = nc.dram_tensor("out", (R_CORE, C, POOL, POOL), f32,
                           kind="ExternalOutput")

    with tile.TileContext(nc) as tc:
        _body(tc, nc, img_in, idx_in, wr_in, ones_in, out_t, f32, f16, i16)

    nc.compile()
    return nc


def _body(tc, nc, img_in, idx_in, wr_in, ones_in, out_t, f32, f16, i16):
    from contextlib import ExitStack
    ctx = ExitStack()
    with ctx:
        const_pool = ctx.enter_context(tc.tile_pool(name="const", bufs=1))
        g_pool = ctx.enter_context(tc.tile_pool(name="gather", bufs=3))
        wr_pool = ctx.enter_context(tc.tile_pool(name="wrow", bufs=2))
        w_pool = ctx.enter_context(tc.tile_pool(name="wts", bufs=3))
        o_pool = ctx.enter_context(tc.tile_pool(name="outs", bufs=2))
        o32_pool = ctx.enter_context(tc.tile_pool(name="outs32", bufs=2))
        psum_pool = ctx.enter_context(
            tc.tile_pool(name="psum", bufs=2, space="PSUM"))

        # batch-0 idx first so the first gather's DGE starts ASAP
        spb0 = 4 * BATCHES[0][2] // 16
        idxs = const_pool.tile([128, IDX_COLS], i16, tag="idx")
        nc.sync.dma_start(idxs[:, 0:spb0], idx_in.ap()[:, 0:spb0])
        nc.sync.dma_start(idxs[:, spb0:], idx_in.ap()[:, spb0:])
        ones16 = const_pool.tile([1, 128], f16, tag="ones")
        nc.gpsimd.memset(ones16[:, :], 1.0)

        icol = 0
        woff = 0
        for b, (rb0, nr, nidxc) in enumerate(BATCHES):
            valc = nr * PTS
            spc = nidxc // 16
            # max-size tiles, view-carved per batch so all batches share tags
            tkf = g_pool.tile([128, 4 * 4 * NXM], f16, tag="T")
            for k in range(4):
                dst = tkf[:, k * 4 * nidxc:(k + 1) * 4 * nidxc].rearrange(
                    "p (j i) -> p j i", j=4)
                nc.gpsimd.dma_gather(
                    dst, img_in.ap()[:, :],
                    idxs[:, icol + k * spc:icol + (k + 1) * spc],
                    nidxc, nidxc, C,
                    transpose=True,
                )
            wrow = wr_pool.tile([1, 4 * NXM], f16, tag="wr")
            nc.sync.dma_start(wrow[:, 0:4 * nidxc],
                              wr_in.ap()[:, woff:woff + 4 * nidxc])
            obf = o_pool.tile([128, 4 * RB_MAX * PTS], f16, tag="ob")
            ob = obf[:, 0:4 * valc].rearrange("p (j i) -> p j i", j=4)
            for k in range(4):
                ps = psum_pool.tile([128, NXM], f32, tag="ps")
                for m0 in range(0, nidxc, 512):
                    m1 = min(m0 + 512, nidxc)
                    nc.tensor.matmul(
                        ps[:, m0:m1], ones16[:, :],
                        wrow[:, k * nidxc + m0:k * nidxc + m1],
                        start=True, stop=True)
                wk = w_pool.tile([128, NXM], f16, tag="W")
                nc.scalar.copy(wk[:, 0:nidxc], ps[:, 0:nidxc])
                wkb = wk[:, 0:valc].unsqueeze(1).broadcast_to(
                    [128, 4, valc])
                tkk = tkf[:, k * 4 * nidxc:(k + 1) * 4 * nidxc].rearrange(
                    "p (j i) -> p j i", j=4)[:, :, 0:valc]
                if k == 0:
                    nc.vector.tensor_mul(ob[:, :, :], tkk, wkb)
                else:
                    nc.vector.tensor_mul(tkk, tkk, wkb)
                    nc.vector.tensor_add(ob[:, :, :], ob[:, :, :], tkk)

            # fp16 -> f32 cast into (r, j, s) layout on Act
            o32f = o32_pool.tile([128, RB_MAX * 4 * PTS], f32, tag="o32")
            ob32 = o32f[:, 0:nr * 4 * PTS].rearrange(
                "p (r q) -> p r q", r=nr)
            halves = ((0, nr),) if nr <= 9 else ((0, nr // 2), (nr // 2, nr))
            for r0, r1 in halves:
                nc.scalar.copy(
                    ob32[:, r0:r1, :].rearrange("p r (j s) -> p r j s", j=4),
                    ob[:, :, r0 * PTS:r1 * PTS].rearrange(
                        "p j (r s) -> p r j s", r=r1 - r0))
                dste = out_t.ap()[rb0 + r0:rb0 + r1, :, :, :].rearrange(
                    "r (p j) py px -> p r (j py px)", j=4)
                nc.sync.dma_start(dste, ob32[:, r0:r1, :])
            icol += 4 * spc
            woff += 4 * nidxc


def _get_program():
    if "nc" not in _prog_cache:
        _prog_cache["nc"] = _build_program()
    return _prog_cache["nc"]


def _prep_image(img_n):
    """img_n (512, 64, 64) f32 -> [hw, pi(c)] fp16 row-major."""
    t = np.arange(C)
    perm = 4 * (t % 128) + t // 128      # position t holds channel perm[t]
    rows = img_n.reshape(C, HW).T        # [hw, c]
    return np.ascontiguousarray(rows[:, perm].astype(np.float16))


def _prep_rois(rois_half):
    """rois_half (150, 4) f32 -> (idxw [128, NB*SPB] i16,
    wrows [4, NB*NIDXC] f16). All arithmetic in f32 to match reference."""
    f = np.float32
    bx = rois_half.astype(f)
    y1 = bx[:, 0] / f(IH - 1.0)
    x1 = bx[:, 1] / f(IW - 1.0)
    y2 = bx[:, 2] / f(IH - 1.0)
    x2 = bx[:, 3] / f(IW - 1.0)
    g = (np.arange(POOL, dtype=f) / f(POOL - 1.0)).astype(f)
    in_y = ((y1[:, None] + (y2 - y1)[:, None] * g) * f(H - 1.0)).astype(f)
    in_x = ((x1[:, None] + (x2 - x1)[:, None] * g) * f(W - 1.0)).astype(f)

    def axis(inn, hi):
        val = ((inn >= 0.0) & (inn <= hi)).astype(f)
        c0f = np.floor(inn)
        c0 = np.clip(c0f, 0, hi).astype(np.int32)
        cb = np.minimum(c0 + 1, int(hi))
        lc = (inn - c0f).astype(f)
        wa = ((f(1.0) - lc) * val).astype(f)
        wb = (lc * val).astype(f)
        return c0, cb, wa, wb

    y0, yb, wya, wyb = axis(in_y, H - 1.0)
    x0, xb, wxa, wxb = axis(in_x, W - 1.0)

    idxw = np.zeros((128, IDX_COLS), np.int16)
    wrows = np.zeros((1, WR_LEN), np.float16)
    corners = ((y0, x0, wya, wxa), (y0, xb, wya, wxb),
               (yb, x0, wyb, wxa), (yb, xb, wyb, wxb))
    idx_fulls = []
    w_fulls = []
    for yc, xc, wy, wx in corners:
        idx_fulls.append(
            (yc[:, :, None] * W + xc[:, None, :]).reshape(R_CORE, PTS))
        w_fulls.append((wy[:, :, None] * wx[:, None, :]).astype(f).reshape(
            R_CORE, PTS))
    icol = 0
    woff = 0
    for rb0, nr, nidxc in BATCHES:
        valc = nr * PTS
        for k in range(4):
            flat = np.zeros(nidxc, np.int32)
            flat[:valc] = idx_fulls[k][rb0:rb0 + nr].reshape(-1)
            wrapped = flat.reshape(nidxc // 16, 16).T.astype(np.int16)
            spc = nidxc // 16
            idxw[:, icol + k * spc:icol + (k + 1) * spc] = \
                np.tile(wrapped, (8, 1))
            wrows[0, woff + k * nidxc:woff + k * nidxc + valc] = \
                w_fulls[k][rb0:rb0 + nr].reshape(-1).astype(np.float16)
        icol += 4 * spc
        woff += 4 * nidxc
    return idxw, wrows


def _make_in_maps(img, rois):
    ones = np.ones((1, 128), np.float16)
    img_pm = {}
    in_maps = []
    for c in range(N_CORES):
        n, half = c // 2, c % 2
        if n not in img_pm:
            img_pm[n] = _prep_image(img[n])
        idxw, wrows = _prep_rois(
            rois[n, half * R_CORE:(half + 1) * R_CORE])
        in_maps.append({
            "img": img_pm[n],
            "idx": idxw,
            "wrows": wrows,
            "ones": ones,
        })
    return in_maps


def kernel(img: np.ndarray, rois: np.ndarray,
           input_image: np.ndarray) -> np.ndarray:
    from concourse.bass_utils import run_bass_kernel_spmd

    nc = _get_program()
    in_maps = _make_in_maps(np.asarray(img, dtype=np.float32),
                            np.asarray(rois, dtype=np.float32))
    res = run_bass_kernel_spmd(nc, in_maps, core_ids=list(range(N_CORES)))
    out = np.empty((N, B, C, POOL, POOL), dtype=np.float32)
    for c in range(N_CORES):
        n, half = c // 2, c % 2
        out[n, half * R_CORE:(half + 1) * R_CORE] = res.results[c]["out"]
    return out


# revision 18
# speedup vs baseline: 1.0021x; 1.0021x over previous
"""CropAndResize (tf.image.crop_and_resize semantics, bilinear, extrap=0)
Trainium2 Bass kernel, data-parallel over 8 NeuronCores.

Full inputs:  img (4,512,64,64) f32, rois (4,300,4) f32, input_image (4,3,1024,1024) f32
Full output:  (4,300,512,7,7) f32

Sharding: core c handles image n = c//2 and roi slice
[(c%2)*150 : (c%2)*150+150].

Host prep (per core, cheap O(KB) numpy on the 4-number-per-roi boxes):
  - img is transposed to row-major [hw, c] fp16 with a channel permutation
    pi(c) = (c//4) + 128*(c%4), so the transpose-mode dma_gather lands
    channel 4p+j on partition p, slot j. That makes the final output DMA
    descriptor (j,py,px) = 784B contiguous (full DMA bandwidth, no <512B
    penalty).
  - bilinear corner indices (wrapped int16 [16,*] layout, replicated to 128
    partitions) and fp16 corner weights (dense j = r*49+pt rows for the PE
    ones-broadcast) are computed from the rois in f32, matching the
    reference arithmetic step for step.

Device program per batch (sizes 18x8 + 6 rois; per-corner num_idxs is
n*49 padded to a multiple of 128, kept under the ~1024-idx dma_gather cap):
  1. four per-corner dma_gathers (1KiB rows) from DRAM img.
  2. per corner: PE ones-matmul broadcasts the weight row to 128
     partitions (PSUM), Act copies PSUM -> fp16 SBUF.
  3. DVE blend: ob[j] = sum_k T_k[j] * w_k  (fp16, 2x DVE mode, weight
     broadcast across the 4 channel slots via a stride-0 free dim).
  4. Act casts fp16 -> f32 into the (r, j, s) output layout.
  5. one HWDGE DMA per half-batch writes out[r, 4p+j, py, px]
     (784B descriptors, full DMA bandwidth).
"""

import os
import sys

import numpy as np

_RL_REPO_CANDIDATES = ["/opt/trn_rl_repo", "/root/.axon_site/_ro/trn_rl_repo"]
for _p in _RL_REPO_CANDIDATES:
    if os.path.isdir(_p) and _p not in sys.path:
        sys.path.insert(0, _p)

# ---------------------------------------------------------------- constants
N_CORES = 8
N, C, H, W = 4, 512, 64, 64
B = 300
POOL = 7
PTS = POOL * POOL      # 49
IH, IW = 1024.0, 1024.0
HW = H * W             # 4096
R_CORE = B // 2        # 150 rois per core


def _mk_batches():
    # (roi_start, n_rois, nidxc): per-corner idx count is n*49 padded to a
    # multiple of 128; the dma_gather HW caps num_idxs below 1024, so 18
    # rois (882 -> 896) is the densest clean batch. 8x18 + 6 = 150.
    out = []
    r0 = 0
    for nr in [18] * 8 + [6]:
        nidxc = -(-(nr * PTS) // 128) * 128
        out.append((r0, nr, nidxc))
        r0 += nr
    return out


BATCHES = _mk_batches()
NB = len(BATCHES)
IDX_COLS = sum(4 * nx // 16 for _, _, nx in BATCHES)   # wrapped idx columns
WR_LEN = sum(4 * nx for _, _, nx in BATCHES)           # weight row length
RB_MAX = max(nr for _, nr, _ in BATCHES)
NXM = max(nx for _, _, nx in BATCHES)   # largest per-corner idx count

_prog_cache = {}


def _build_program():
    import concourse.bass as bass
    import concourse.bacc as bacc
    import concourse.mybir as mybir
    import concourse.tile as tile

    f32 = mybir.dt.float32
    f16 = mybir.dt.float16
    i16 = mybir.dt.int16

    nc = bacc.Bacc("TRN2", target_bir_lowering=False, debug=False,
                   num_devices=N_CORES)

    img_in = nc.dram_tensor("img", (HW, C), f16, kind="ExternalInput")
    idx_in = nc.dram_tensor("idx", (128, IDX_COLS), i16, kind="ExternalInput")
    wr_in = nc.dram_tensor("wrows", (1, WR_LEN), f16, kind="ExternalInput")
    ones_in = nc.dram_tensor("ones", (1, 128), f16, kind="ExternalInput")
    out_t = nc.dram_tensor("out", (R_CORE, C, POOL, POOL), f32,
                           kind="ExternalOutput")

    with tile.TileContext(nc) as tc:
        _body(tc, nc, img_in, idx_in, wr_in, ones_in, out_t, f32, f16, i16)

    nc.compile()
    return nc


def _body(tc, nc, img_in, idx_in, wr_in, ones_in, out_t, f32, f16, i16):
    from contextlib import ExitStack
    ctx = ExitStack()
    with ctx:
        const_pool = ctx.enter_context(tc.tile_pool(name="const", bufs=1))
        g_pool = ctx.enter_context(tc.tile_pool(name="gather", bufs=3))
        wr_pool = ctx.enter_context(tc.tile_pool(name="wrow", bufs=2))
        w_pool = ctx.enter_context(tc.tile_pool(name="wts", bufs=3))
        o_pool = ctx.enter_context(tc.tile_pool(name="outs", bufs=2))
        o32_pool = ctx.enter_context(tc.tile_pool(name="outs32", bufs=2))
        psum_pool = ctx.enter_context(
            tc.tile_pool(name="psum", bufs=2, space="PSUM"))

        # batch-0 corner-0 idx first so the first gather's DGE starts ASAP
        spc0 = BATCHES[0][2] // 16
        spb0 = 4 * spc0
        idxs = const_pool.tile([128, IDX_COLS], i16, tag="idx")
        nc.sync.dma_start(idxs[:, 0:spc0], idx_in.ap()[:, 0:spc0])
        nc.sync.dma_start(idxs[:, spc0:spb0], idx_in.ap()[:, spc0:spb0])
        nc.sync.dma_start(idxs[:, spb0:], idx_in.ap()[:, spb0:])
        ones16 = const_pool.tile([1, 128], f16, tag="ones")
        nc.gpsimd.memset(ones16[:, :], 1.0)

        icol = 0
        woff = 0
        for b, (rb0, nr, nidxc) in enumerate(BATCHES):
            valc = nr * PTS
            spc = nidxc // 16
            # weight row first: its PE/Act broadcast chain overlaps the
            # gather transfers, so the k=0 blend can start when gather-0 lands
            wrow = wr_pool.tile([1, 4 * NXM], f16, tag="wr")
            nc.sync.dma_start(wrow[:, 0:4 * nidxc],
                              wr_in.ap()[:, woff:woff + 4 * nidxc])
            # max-size tiles, view-carved per batch so all batches share tags
            tkf = g_pool.tile([128, 4 * 4 * NXM], f16, tag="T")
            for k in range(4):
                dst = tkf[:, k * 4 * nidxc:(k + 1) * 4 * nidxc].rearrange(
                    "p (j i) -> p j i", j=4)
                nc.gpsimd.dma_gather(
                    dst, img_in.ap()[:, :],
                    idxs[:, icol + k * spc:icol + (k + 1) * spc],
                    nidxc, nidxc, C,
                    transpose=True,
                )
            obf = o_pool.tile([128, 4 * RB_MAX * PTS], f16, tag="ob")
            ob = obf[:, 0:4 * valc].rearrange("p (j i) -> p j i", j=4)
            for k in range(4):
                ps = psum_pool.tile([128, NXM], f32, tag="ps")
                for m0 in range(0, nidxc, 512):
                    m1 = min(m0 + 512, nidxc)
                    nc.tensor.matmul(
                        ps[:, m0:m1], ones16[:, :],
                        wrow[:, k * nidxc + m0:k * nidxc + m1],
                        start=True, stop=True)
                wk = w_pool.tile([128, NXM], f16, tag="W")
                nc.scalar.copy(wk[:, 0:nidxc], ps[:, 0:nidxc])
                wkb = wk[:, 0:valc].unsqueeze(1).broadcast_to(
                    [128, 4, valc])
                tkk = tkf[:, k * 4 * nidxc:(k + 1) * 4 * nidxc].rearrange(
                    "p (j i) -> p j i", j=4)[:, :, 0:valc]
                if k == 0:
                    nc.vector.tensor_mul(ob[:, :, :], tkk, wkb)
                else:
                    nc.vector.tensor_mul(tkk, tkk, wkb)
                    nc.vector.tensor_add(ob[:, :, :], ob[:, :, :], tkk)

            # fp16 -> f32 cast into (r, j, s) layout on Act
            o32f = o32_pool.tile([128, RB_MAX * 4 * PTS], f32, tag="o32")
            ob32 = o32f[:, 0:nr * 4 * PTS].rearrange(
                "p (r q) -> p r q", r=nr)
            halves = ((0, nr),) if nr <= 9 else ((0, nr // 2), (nr // 2, nr))
            for r0, r1 in halves:
                nc.scalar.copy(
                    ob32[:, r0:r1, :].rearrange("p r (j s) -> p r j s", j=4),
                    ob[:, :, r0 * PTS:r1 * PTS].rearrange(
                        "p j (r s) -> p r j s", r=r1 - r0))
                dste = out_t.ap()[rb0 + r0:rb0 + r1, :, :, :].rearrange(
                    "r (p j) py px -> p r (j py px)", j=4)
                nc.sync.dma_start(dste, ob32[:, r0:r1, :])
            icol += 4 * spc
            woff += 4 * nidxc


def _get_program():
    if "nc" not in _prog_cache:
        _prog_cache["nc"] = _build_program()
    return _prog_cache["nc"]


def _prep_image(img_n):
    """img_n (512, 64, 64) f32 -> [hw, pi(c)] fp16 row-major."""
    t = np.arange(C)
    perm = 4 * (t % 128) + t // 128      # position t holds channel perm[t]
    rows = img_n.reshape(C, HW).T        # [hw, c]
    return np.ascontiguousarray(rows[:, perm].astype(np.float16))


def _prep_rois(rois_half):
    """rois_half (150, 4) f32 -> (idxw [128, NB*SPB] i16,
    wrows [4, NB*NIDXC] f16). All arithmetic in f32 to match reference."""
    f = np.float32
    bx = rois_half.astype(f)
    y1 = bx[:, 0] / f(IH - 1.0)
    x1 = bx[:, 1] / f(IW - 1.0)
    y2 = bx[:, 2] / f(IH - 1.0)
    x2 = bx[:, 3] / f(IW - 1.0)
    g = (np.arange(POOL, dtype=f) / f(POOL - 1.0)).astype(f)
    in_y = ((y1[:, None] + (y2 - y1)[:, None] * g) * f(H - 1.0)).astype(f)
    in_x = ((x1[:, None] + (x2 - x1)[:, None] * g) * f(W - 1.0)).astype(f)

    def axis(inn, hi):
        val = ((inn >= 0.0) & (inn <= hi)).astype(f)
        c0f = np.floor(inn)
        c0 = np.clip(c0f, 0, hi).astype(np.int32)
        cb = np.minimum(c0 + 1, int(hi))
        lc = (inn - c0f).astype(f)
        wa = ((f(1.0) - lc) * val).astype(f)
        wb = (lc * val).astype(f)
        return c0, cb, wa, wb

    y0, yb, wya, wyb = axis(in_y, H - 1.0)
    x0, xb, wxa, wxb = axis(in_x, W - 1.0)

    idxw = np.zeros((128, IDX_COLS), np.int16)
    wrows = np.zeros((1, WR_LEN), np.float16)
    corners = ((y0, x0, wya, wxa), (y0, xb, wya, wxb),
               (yb, x0, wyb, wxa), (yb, xb, wyb, wxb))
    idx_fulls = []
    w_fulls = []
    for yc, xc, wy, wx in corners:
        idx_fulls.append(
            (yc[:, :, None] * W + xc[:, None, :]).reshape(R_CORE, PTS))
        w_fulls.append((wy[:, :, None] * wx[:, None, :]).astype(f).reshape(
            R_CORE, PTS))
    icol = 0
    woff = 0
    for rb0, nr, nidxc in BATCHES:
        valc = nr * PTS
        for k in range(4):
            flat = np.zeros(nidxc, np.int32)
            flat[:valc] = idx_fulls[k][rb0:rb0 + nr].reshape(-1)
            wrapped = flat.reshape(nidxc // 16, 16).T.astype(np.int16)
            spc = nidxc // 16
            idxw[:, icol + k * spc:icol + (k + 1) * spc] = \
                np.tile(wrapped, (8, 1))
            wrows[0, woff + k * nidxc:woff + k * nidxc + valc] = \
                w_fulls[k][rb0:rb0 + nr].reshape(-1).astype(np.float16)
        icol += 4 * spc
        woff += 4 * nidxc
    return idxw, wrows


def _make_in_maps(img, rois):
    ones = np.ones((1, 128), np.float16)
    img_pm = {}
    in_maps = []
    for c in range(N_CORES):
        n, half = c // 2, c % 2
        if n not in img_pm:
            img_pm[n] = _prep_image(img[n])
        idxw, wrows = _prep_rois(
            rois[n, half * R_CORE:(half + 1) * R_CORE])
        in_maps.append({
            "img": img_pm[n],
            "idx": idxw,
            "wrows": wrows,
            "ones": ones,
        })
    return in_maps


def kernel(img: np.ndarray, rois: np.ndarray,
           input_image: np.ndarray) -> np.ndarray:
    from concourse.bass_utils import run_bass_kernel_spmd

    nc = _get_program()
    in_maps = _make_in_maps(np.asarray(img, dtype=np.float32),
                            np.asarray(rois, dtype=np.float32))
    res = run_bass_kernel_spmd(nc, in_maps, core_ids=list(range(N_CORES)))
    out = np.empty((N, B, C, POOL, POOL), dtype=np.float32)
    for c in range(N_CORES):
        n, half = c // 2, c % 2
        out[n, half * R_CORE:(half + 1) * R_CORE] = res.results[c]["out"]
    return out
